# revision 1
# baseline (speedup 1.0000x reference)
"""Sharded brute-force kNN (cosine-sim top-k) on 8 Trainium2 NeuronCores.

Strategy (passage-row-wise sharding, MODE="fp8dr"):
  - Each core gets a 32768-passage shard (of 262144) plus the full 2048
    queries, both cast to fp8e4m3 and pre-transposed host-side to K-major
    layout (25MB + 2MB per core).
  - Device: S = Q @ P_shard.T as fp8 DoubleRow matmuls (2 fp8 MACs per PE
    cell per cycle -> the PE streams one [128q, 512p] PSUM bank per 3
    matmuls of K=256 each; measured 216ns/matmul, ~93% of the fp8-DR
    compute roofline). ACT and DVE alternate casting each PSUM bank to
    fp8 in SBUF; every 8 chunks one DMA writes the staged [128, 4096]
    block to a dense [2048, 32768] fp8 sims output (67MB/core).
  - Host: maps fp8 sims to order-preserving uint8 keys, cuts each row at
    a sampled per-row threshold (~1.5k survivors; the exact top-k is a
    subset with tens-of-sigma margin vs the ~1-sigma fp8 matmul noise),
    rescores every survivor exactly in fp32 (BLAS gemv), and takes the
    exact top-k with jax.lax.top_k tie-breaking (lowest index first).

Older modes kept for fallback: "f32r" (TF32-class matmul + on-device DVE
max8/find_index8 per-512-chunk top-8, exact host rescore of a top-128
cut) and "f32" (native fp32 matmul, 4 cycles/row, no rescore needed).
"""
import os
import time as _time

import numpy as np

import concourse.bacc as bacc
import concourse.tile as tile
from concourse import mybir
from concourse.bass_utils import run_bass_kernel_spmd

P = 128
Q = 2048              # queries (replicated on all cores)
D = 768               # embedding dim = 6 k-tiles of 128
NCORES = 8
NTOTAL = 262144       # total passages
NSH = NTOTAL // NCORES  # 32768 passages per core
CHUNK = 512           # passages per PSUM bank
NCHUNK = NSH // CHUNK  # 64
NQB = Q // P          # 16 query blocks
KT = D // P           # 6 k-tiles

TRACE = False         # set True (e.g. from test.py) to capture an NTFF profile
LAST_PERF = None      # BassKernelResults of the last run when TRACE was set

MODE = "fp8dr"        # "fp8dr": fp8 DoubleRow matmul, full bf16 sims to DRAM,
                      #          host does the top-k scan + exact fp32 rescore
                      # "f32r": TF32-class matmul + device max8 + exact host rescore
                      # "f32": native fp32 matmul (4 cycles/row), no rescore needed
RESCORE = 128         # candidates per row rescored exactly on host (f32r mode)
RESCORE8 = 1024       # fallback top-C in fp8dr mode (fp8 keys are coarse)
SBUF_MAX8 = False     # bounce sims PSUM->SBUF on the idle ACT engine and run
                      # the DVE max8/find_index8 scans from SBUF (faster reads)

_NC_CACHE = {}


def _build(mode):
    mm_dt = mybir.dt.float32 if mode == "f32" else mybir.dt.float32r
    nc = bacc.Bacc("TRN2", target_bir_lowering=False)
    qt = nc.dram_tensor("qt", [D, Q], mybir.dt.float32, kind="ExternalInput")
    pt = nc.dram_tensor("pt", [D, NSH], mybir.dt.float32, kind="ExternalInput")
    vals = nc.dram_tensor("vals", [Q, NCHUNK * 8], mybir.dt.float32, kind="ExternalOutput")
    idx = nc.dram_tensor("idx", [Q, NCHUNK * 8], mybir.dt.uint32, kind="ExternalOutput")

    qt_ap = qt.ap().rearrange("(s p) q -> p s q", p=P)   # [128, 6, 2048]
    pt_ap = pt.ap().rearrange("(s p) n -> p s n", p=P)   # [128, 6, 32768]
    if mm_dt != mybir.dt.float32:
        qt_ap = qt_ap.bitcast(mm_dt)
        pt_ap = pt_ap.bitcast(mm_dt)

    with tile.TileContext(nc) as tc:
        with (
            tc.tile_pool(name="qpool", bufs=1) as qpool,
            tc.tile_pool(name="ppool", bufs=3) as ppool,
            tc.tile_pool(name="opool", bufs=1) as opool,
            tc.tile_pool(name="cpool", bufs=4) as cpool,
            tc.tile_pool(name="pspool", bufs=8, space="PSUM") as pspool,
        ):
            qt_t = qpool.tile([P, KT, Q], mm_dt, name="qt_t")
            nc.sync.dma_start(qt_t[:], qt_ap)

            ovals = [opool.tile([P, NCHUNK * 8], mybir.dt.float32, tag=f"ov{b}", name=f"ov{b}")
                     for b in range(NQB)]
            oidx = [opool.tile([P, NCHUNK * 8], mybir.dt.uint32, tag=f"oi{b}", name=f"oi{b}")
                    for b in range(NQB)]

            for c in range(NCHUNK):
                pt_t = ppool.tile([P, KT, CHUNK], mm_dt, tag="pt", name="pt_t")
                nc.sync.dma_start(pt_t[:], pt_ap[:, :, c * CHUNK:(c + 1) * CHUNK])
                for b in range(NQB):
                    ps = pspool.tile([P, CHUNK], mybir.dt.float32, tag="ps", name="ps")
                    for k in range(KT):
                        nc.tensor.matmul(
                            ps[:], qt_t[:, k, b * P:(b + 1) * P], pt_t[:, k],
                            start=(k == 0), stop=(k == KT - 1),
                        )
                    if SBUF_MAX8:
                        sc = cpool.tile([P, CHUNK], mybir.dt.float32, tag="sc", name="sc")
                        nc.scalar.copy(sc[:], ps[:])
                        src = sc[:]
                    else:
                        src = ps[:]
                    v8 = ovals[b][:, c * 8:(c + 1) * 8]
                    nc.vector.max(v8, src)
                    nc.vector.max_index(oidx[b][:, c * 8:(c + 1) * 8], v8, src)

            for b in range(NQB):
                nc.sync.dma_start(vals.ap()[b * P:(b + 1) * P], ovals[b][:])
                nc.sync.dma_start(idx.ap()[b * P:(b + 1) * P], oidx[b][:])
    nc.compile()
    return nc


def _build_fp8():
    FP8 = mybir.dt.float8e4
    nc = bacc.Bacc("TRN2", target_bir_lowering=False)
    qt = nc.dram_tensor("qt", [D, Q], FP8, kind="ExternalInput")
    pt = nc.dram_tensor("pt", [D, NSH], FP8, kind="ExternalInput")
    sims = nc.dram_tensor("sims", [Q, NSH], FP8, kind="ExternalOutput")

    qt_ap = qt.ap().rearrange("(s p) q -> p s q", p=P)   # [128, 6, 2048]
    pt_ap = pt.ap().rearrange("(s p) n -> p s n", p=P)   # [128, 6, 32768]

    with tile.TileContext(nc) as tc:
        with (
            tc.tile_pool(name="qpool", bufs=1) as qpool,
            tc.tile_pool(name="ppool", bufs=4) as ppool,
            tc.tile_pool(name="cpool", bufs=1) as cpool,
            tc.tile_pool(name="pspool", bufs=8, space="PSUM") as pspool,
        ):
            qt_t = qpool.tile([P, KT, Q], FP8, name="qt_t")
            nc.sync.dma_start(qt_t[:], qt_ap)

            G = 8  # chunks per staged output DMA
            stage = [cpool.tile([P, G * CHUNK], FP8, tag=f"st{b}", name=f"st{b}")
                     for b in range(NQB)]

            for c in range(NCHUNK):
                pt_t = ppool.tile([P, KT, CHUNK], FP8, tag="pt", name="pt_t")
                nc.sync.dma_start(pt_t[:], pt_ap[:, :, c * CHUNK:(c + 1) * CHUNK])
                g = c % G
                for b in range(NQB):
                    ps = pspool.tile([P, CHUNK], mybir.dt.float32, tag="ps", name="ps")
                    for k in range(KT // 2):
                        nc.tensor.matmul(
                            ps[:], qt_t[:, 2 * k:2 * k + 2, b * P:(b + 1) * P],
                            pt_t[:, 2 * k:2 * k + 2, :],
                            start=(k == 0), stop=(k == KT // 2 - 1),
                            perf_mode=mybir.MatmulPerfMode.DoubleRow,
                        )
                    sc = stage[b][:, g * CHUNK:(g + 1) * CHUNK]
                    # alternate the PSUM->SBUF cast copy between ACT and DVE
                    if (c * NQB + b) % 2 == 0:
                        nc.scalar.copy(sc, ps[:])
                    else:
                        nc.vector.tensor_copy(sc, ps[:])
                    if g == G - 1:
                        nc.sync.dma_start(
                            sims.ap()[b * P:(b + 1) * P,
                                      (c - G + 1) * CHUNK:(c + 1) * CHUNK],
                            stage[b][:])
    nc.compile()
    return nc


def _fp8_sort_keys(a):
    """Order-preserving fp8 -> uint8 map (no NaNs expected)."""
    u = a.view(np.uint8)
    flip = (u >> 7) * np.uint8(0x7F) + np.uint8(0x80)
    return u ^ flip


def _t(label, t0):
    if os.environ.get("KNN_TIMING"):
        print(f"[knn] {label}: {_time.time() - t0:.2f}s", flush=True)
    return _time.time()


def kernel(query_embed, passage_embed, top_k):
    global LAST_PERF, _NC_CACHE
    t0 = _time.time()
    q = np.ascontiguousarray(np.asarray(query_embed, dtype=np.float32))
    p = np.asarray(passage_embed, dtype=np.float32)
    k = int(top_k)
    assert q.shape == (Q, D) and p.shape == (NTOTAL, D), (q.shape, p.shape)
    assert 1 <= k <= 128, k

    if MODE not in _NC_CACHE:
        _NC_CACHE[MODE] = _build_fp8() if MODE == "fp8dr" else _build(MODE)
    nc = _NC_CACHE[MODE]
    t0 = _t("build", t0)

    if MODE == "fp8dr":
        NP8 = mybir.dt.np(mybir.dt.float8e4)
        q8 = q.astype(NP8)
        p8 = p.astype(NP8)
        qt = np.ascontiguousarray(q8.T)
        in_maps = [
            {"qt": qt, "pt": np.ascontiguousarray(p8[c * NSH:(c + 1) * NSH].T)}
            for c in range(NCORES)
        ]
    else:
        qt = np.ascontiguousarray(q.T)
        in_maps = [
            {"qt": qt, "pt": np.ascontiguousarray(p[c * NSH:(c + 1) * NSH].T)}
            for c in range(NCORES)
        ]
    t0 = _t("input prep", t0)
    out = run_bass_kernel_spmd(nc, in_maps, core_ids=list(range(NCORES)), trace=TRACE)
    if TRACE:
        LAST_PERF = out
    t0 = _t("device run (incl neff compile + transfers)", t0)

    if MODE == "fp8dr":
        keys = np.empty((Q, NTOTAL), dtype=np.uint8)
        for c in range(NCORES):
            keys[:, c * NSH:(c + 1) * NSH] = _fp8_sort_keys(out.results[c]["sims"])
        t0 = _t("keys", t0)
        # Per-row key threshold from a column sample, aiming for ~1.5*RESCORE8
        # survivors; statistically the true top-k is always a subset (fp8
        # matmul noise sigma ~1, fp8 key buckets ~8 wide at the boundary, vs
        # tens-of-sigma margins in the cut). Rows whose survivor count lands
        # low get an exact top-RESCORE8 fallback.
        m = RESCORE8
        S = NTOTAL // 16
        skth = (3 * m) // (2 * 16)
        th = np.partition(keys[:, :S], S - skth, axis=1)[:, S - skth]
        mask = keys >= th[:, None]
        counts = mask.sum(axis=1)
        bad = np.nonzero((counts < max(2 * k, 256)) | (counts > 16 * m))[0]
        for r in bad:  # rare (sampling tail); re-cut the row at its exact
            # m-th largest key, tie-inclusive so equal keys are all kept
            th_r = np.partition(keys[r], NTOTAL - m)[NTOTAL - m]
            mask[r] = keys[r] >= th_r
        t0 = _t("threshold scan", t0)
        rows, cols = np.nonzero(mask)
        row_starts = np.searchsorted(rows, np.arange(Q + 1))
        t0 = _t("survivors", t0)
        # exact fp32 rescore of every survivor (per-row BLAS gemv)
        exact = np.empty(len(cols), dtype=np.float32)
        for r in range(Q):
            s0, e0 = row_starts[r], row_starts[r + 1]
            exact[s0:e0] = p[cols[s0:e0]] @ q[r]
        t0 = _t("rescore", t0)
        # ties -> lowest passage index, matching jax.lax.top_k
        order = np.lexsort((cols, -exact, rows))
        cols = cols[order]
        exact = exact[order]
        pick = (row_starts[:-1, None] + np.arange(k)[None, :]).ravel()
        inds = cols[pick].reshape(Q, k).astype(np.int32)
        vals = exact[pick].reshape(Q, k)
        t0 = _t("final sort", t0)
        return inds, vals

    # merge candidates: [Q, 8*512] values and global indices
    cand_vals = np.concatenate([out.results[c]["vals"] for c in range(NCORES)], axis=1)
    base = (np.arange(NCHUNK, dtype=np.int64)[:, None] * CHUNK).reshape(1, NCHUNK, 1)
    cand_idx = np.concatenate(
        [
            (out.results[c]["idx"].astype(np.int64).reshape(Q, NCHUNK, 8) + base
             + c * NSH).reshape(Q, NCHUNK * 8)
            for c in range(NCORES)
        ],
        axis=1,
    )
    # exact stable top-k: descending value, ties -> lowest passage index.
    # cand arrays are index-ordered among equal values (chunk-major layout,
    # and max_index assigns ascending indices to within-chunk duplicates),
    # so a stable sort on -value reproduces jax.lax.top_k tie-breaking.
    if MODE == "f32":
        sel = np.argsort(-cand_vals, axis=1, kind="stable")[:, :k]
        inds = np.take_along_axis(cand_idx, sel, axis=1).astype(np.int32)
        vals = np.take_along_axis(cand_vals, sel, axis=1)
        return inds, vals

    # f32r mode: device values are TF32-class. Take a top-RESCORE cut by
    # device value (stable; huge margin vs the TF32 noise), recompute those
    # sims exactly in fp32 on host, and do the final exact top-k.
    m = RESCORE
    sel = np.argsort(-cand_vals, axis=1, kind="stable")[:, :m]
    top_idx = np.take_along_axis(cand_idx, sel, axis=1)        # [Q, m]
    exact = np.empty((Q, m), dtype=np.float32)
    BQ = 256
    for r0 in range(0, Q, BQ):
        r1 = r0 + BQ
        gathered = p[top_idx[r0:r1]]                           # [BQ, m, D]
        exact[r0:r1] = np.einsum("qd,qmd->qm", q[r0:r1], gathered)
    # exact top-k with jax.lax.top_k tie-breaking (ties -> lowest index)
    order = np.lexsort((top_idx, -exact), axis=-1)[:, :k]
    inds = np.take_along_axis(top_idx, order, axis=1).astype(np.int32)
    vals = np.take_along_axis(exact, order, axis=1)
    return inds, vals



# revision 2
# speedup vs baseline: 1.0318x; 1.0318x over previous
"""Sharded brute-force kNN (cosine-sim top-k) on 8 Trainium2 NeuronCores.

Strategy (passage-row-wise sharding, fp8 DoubleRow, passage-stationary):
  - Each core gets a 32768-passage shard (of 262144) plus the full 2048
    queries, both cast to fp8e4m3 host-side (fast bit-twiddling cast) and
    pre-transposed to K-major layout (25MB + 2MB per core).
  - Device: S = P_shard @ Q.T as fp8 DoubleRow matmuls with the PASSAGE
    tile stationary: per 128-passage chunk, 3 k-pair weight loads each
    streamed by 4 query blocks of 512 (12 MMs of 216ns = the fp8-DR
    streaming floor). Output [128, 2048] fp8 per pchunk is cast out of
    PSUM by ACT/DVE alternately and DMA'd every ~2.6us - a uniform
    output stream (no bursts, ~1.5us tail) vs the query-stationary
    variant's 8.4MB end-of-group bursts.
  - Host: maps fp8 sims to order-preserving uint8 keys, cuts each query
    at a sampled threshold (~1.5k survivors; the exact top-k is a subset
    with tens-of-sigma margin vs the ~1-sigma fp8 matmul noise),
    rescores every survivor exactly in fp32 (blocked BLAS GEMM), and
    takes the exact top-k with jax.lax.top_k tie-breaking (lowest index
    first).
"""
import os
import time as _time

import numpy as np

import concourse.bacc as bacc
import concourse.tile as tile
from concourse import mybir
from concourse.bass_utils import run_bass_kernel_spmd

P = 128
Q = 2048              # queries (replicated on all cores)
D = 768               # embedding dim = 6 k-tiles of 128
NCORES = 8
NTOTAL = 262144       # total passages
NSH = NTOTAL // NCORES  # 32768 passages per core
CHUNK = 512           # queries per PSUM bank (moving dim)
NQG = Q // CHUNK      # 4 query groups
NPC = NSH // P        # 256 passage chunks per core
PG = 8                # passage chunks per input DMA group
KT = D // P           # 6 k-tiles

TRACE = False         # set True (e.g. from test.py) to capture an NTFF profile
LAST_PERF = None      # BassKernelResults of the last run when TRACE was set

RESCORE8 = 1024       # target survivor count per query for the host rescore

_NC_CACHE = {}


def _build_fp8():
    FP8 = mybir.dt.float8e4
    nc = bacc.Bacc("TRN2", target_bir_lowering=False)
    qt = nc.dram_tensor("qt", [D, Q], FP8, kind="ExternalInput")
    pt = nc.dram_tensor("pt", [D, NSH], FP8, kind="ExternalInput")
    sims = nc.dram_tensor("sims", [NSH, Q], FP8, kind="ExternalOutput")

    qt_ap = qt.ap().rearrange("(s p) q -> p s q", p=P)   # [128, 6, 2048]
    pt_ap = pt.ap().rearrange("(s p) n -> p s n", p=P)   # [128, 6, 32768]

    DR = mybir.MatmulPerfMode.DoubleRow
    NG = NPC // PG  # input DMA groups

    with tile.TileContext(nc) as tc:
        with (
            tc.tile_pool(name="qpool", bufs=1) as qpool,
            tc.tile_pool(name="ppool", bufs=3) as ppool,
            tc.tile_pool(name="spool", bufs=3) as spool,
            tc.tile_pool(name="pspool", bufs=8, space="PSUM") as pspool,
        ):
            # First passage tile group, then queries one k-pair at a time:
            # the first matmul needs pt group 0 + qt k-pair 0 only, so it
            # can start ~8.8us in instead of waiting for the full 1.5MB
            # query load on the single DMA queue.
            cur = ppool.tile([P, KT, PG * P], FP8, tag="pt", name="pt_t")
            nc.sync.dma_start(cur[:], pt_ap[:, :, 0:PG * P])

            qt_t = qpool.tile([P, KT, Q], FP8, name="qt_t")
            for kk in range(KT // 2):
                nc.sync.dma_start(
                    qt_t[:, 2 * kk:2 * kk + 2, :], qt_ap[:, 2 * kk:2 * kk + 2, :])

            for g in range(NG):
                if g + 1 < NG:
                    nxt = ppool.tile([P, KT, PG * P], FP8, tag="pt", name="pt_t")
                    nc.sync.dma_start(
                        nxt[:], pt_ap[:, :, (g + 1) * PG * P:(g + 2) * PG * P])
                else:
                    nxt = None
                for pi in range(PG):
                    pglob = g * PG + pi
                    st = spool.tile([P, Q], FP8, tag="st", name="st")
                    ps = [pspool.tile([P, CHUNK], mybir.dt.float32, tag="ps", name="ps")
                          for _ in range(NQG)]
                    for kk in range(KT // 2):
                        w = cur[:, 2 * kk:2 * kk + 2, pi * P:(pi + 1) * P]
                        for j in range(NQG):
                            nc.tensor.matmul(
                                ps[j][:], w,
                                qt_t[:, 2 * kk:2 * kk + 2, j * CHUNK:(j + 1) * CHUNK],
                                start=(kk == 0), stop=(kk == KT // 2 - 1),
                                perf_mode=DR,
                            )
                    for j in range(NQG):
                        dst = st[:, j * CHUNK:(j + 1) * CHUNK]
                        # alternate the PSUM->SBUF cast between ACT and DVE
                        if j % 2 == 0:
                            nc.scalar.copy(dst, ps[j][:])
                        else:
                            nc.vector.tensor_copy(dst, ps[j][:])
                    nc.sync.dma_start(
                        sims.ap()[pglob * P:(pglob + 1) * P, :], st[:])
                cur = nxt
    nc.compile()
    return nc


def _f32_to_e4m3_bytes(x):
    """Vectorized fp32 -> fp8e4m3fn (RNE, flush-to-zero below 2^-6).

    ml_dtypes' astype is ~2M elem/s; this is numpy-bit-twiddled and ~50x
    faster. Inputs are N(0,1) so |x| << 240 (no clipping) and the
    subnormal flush adds noise orders of magnitude below the fp8
    quantization noise already tolerated by the host rescore.
    """
    b = np.ascontiguousarray(x, dtype=np.float32).view(np.uint32)
    s = ((b >> 24) & np.uint32(0x80)).astype(np.uint8)
    m = b & np.uint32(0x7FFFFFFF)
    lsb = (m >> np.uint32(20)) & np.uint32(1)
    m = m + np.uint32(0x7FFFF) + lsb
    e8 = (m >> np.uint32(20)).astype(np.int32) - ((127 - 7) << 3)
    out = np.clip(e8, 0, 0x7F).astype(np.uint8) | s
    out[e8 < 8] = s[e8 < 8]  # subnormal/zero -> signed zero
    return out


def _fp8_sort_keys(a):
    """Order-preserving fp8 -> uint8 map (no NaNs expected)."""
    u = a.view(np.uint8)
    flip = (u >> 7) * np.uint8(0x7F) + np.uint8(0x80)
    return u ^ flip


def _t(label, t0):
    if os.environ.get("KNN_TIMING"):
        print(f"[knn] {label}: {_time.time() - t0:.2f}s", flush=True)
    return _time.time()


def kernel(query_embed, passage_embed, top_k):
    global LAST_PERF, _NC_CACHE
    t0 = _time.time()
    q = np.ascontiguousarray(np.asarray(query_embed, dtype=np.float32))
    p = np.asarray(passage_embed, dtype=np.float32)
    k = int(top_k)
    assert q.shape == (Q, D) and p.shape == (NTOTAL, D), (q.shape, p.shape)
    assert 1 <= k <= 128, k

    if "fp8dr" not in _NC_CACHE:
        _NC_CACHE["fp8dr"] = _build_fp8()
    nc = _NC_CACHE["fp8dr"]
    t0 = _t("build", t0)

    NP8 = mybir.dt.np(mybir.dt.float8e4)
    q8 = _f32_to_e4m3_bytes(q).view(NP8)
    p8 = _f32_to_e4m3_bytes(p).view(NP8)
    qt = np.ascontiguousarray(q8.T)
    in_maps = [
        {"qt": qt, "pt": np.ascontiguousarray(p8[c * NSH:(c + 1) * NSH].T)}
        for c in range(NCORES)
    ]
    t0 = _t("input prep", t0)
    out = run_bass_kernel_spmd(nc, in_maps, core_ids=list(range(NCORES)), trace=TRACE)
    if TRACE:
        LAST_PERF = out
    t0 = _t("device run (incl neff compile + transfers)", t0)

    # sims arrive [NSH, Q] per core (passage-major). Work per-core to
    # avoid a 536MB host transpose.
    keys_list = [_fp8_sort_keys(np.asarray(out.results[c]["sims"]))
                 for c in range(NCORES)]
    t0 = _t("keys", t0)
    # Per-query key threshold from a passage sample (first 16384 passages
    # = first half of core 0's shard), aiming for ~1.5*RESCORE8
    # survivors; statistically the true top-k is always a subset (fp8
    # matmul noise sigma ~1, fp8 key buckets ~8 wide at the boundary, vs
    # tens-of-sigma margins in the cut). Queries whose survivor count
    # lands low get an exact top-RESCORE8 fallback.
    m = RESCORE8
    S = NTOTAL // 16
    skth = (3 * m) // (2 * 16)
    th = np.partition(keys_list[0][:S], S - skth, axis=0)[S - skth]  # [Q]
    masks = [kk >= th[None, :] for kk in keys_list]
    counts = masks[0].sum(axis=0, dtype=np.int64)
    for mk in masks[1:]:
        counts += mk.sum(axis=0, dtype=np.int64)
    bad = np.nonzero((counts < max(2 * k, 256)) | (counts > 16 * m))[0]
    for r in bad:  # rare (sampling tail); re-cut the query at its exact
        # m-th largest key, tie-inclusive so equal keys are all kept
        col = np.concatenate([kk[:, r] for kk in keys_list])
        th_r = np.partition(col, NTOTAL - m)[NTOTAL - m]
        for c in range(NCORES):
            masks[c][:, r] = keys_list[c][:, r] >= th_r
    t0 = _t("threshold scan", t0)
    rows_parts, cols_parts = [], []
    for c in range(NCORES):
        pr, qr = np.nonzero(masks[c])
        rows_parts.append(qr)
        cols_parts.append(pr + c * NSH)
    rows = np.concatenate(rows_parts)
    cols = np.concatenate(cols_parts)
    order0 = np.lexsort((cols, rows))  # group by query, cols ascending
    rows = rows[order0]
    cols = cols[order0]
    row_starts = np.searchsorted(rows, np.arange(Q + 1))
    t0 = _t("survivors", t0)
    # exact fp32 rescore of every survivor: blocked GEMM over 16-query
    # stripes (gather survivor passages once, multiply by all 16 queries,
    # select the matching column - BLAS-fast despite the 16x overcompute)
    exact = np.empty(len(cols), dtype=np.float32)
    QB = 16
    for r0 in range(0, Q, QB):
        s0, e0 = row_starts[r0], row_starts[r0 + QB]
        if e0 == s0:
            continue
        sb = p[cols[s0:e0]] @ q[r0:r0 + QB].T          # [ns, QB]
        exact[s0:e0] = sb[np.arange(e0 - s0), rows[s0:e0] - r0]
    t0 = _t("rescore", t0)
    # ties -> lowest passage index, matching jax.lax.top_k
    order = np.lexsort((cols, -exact, rows))
    cols = cols[order]
    exact = exact[order]
    pick = (row_starts[:-1, None] + np.arange(k)[None, :]).ravel()
    inds = cols[pick].reshape(Q, k).astype(np.int32)
    vals = exact[pick].reshape(Q, k)
    t0 = _t("final sort", t0)
    return inds, vals


# revision 8
# speedup vs baseline: 1.0422x; 1.0101x over previous
"""Sharded brute-force kNN (cosine-sim top-k) on 8 Trainium2 NeuronCores.

Strategy (passage-row-wise sharding, fp8 DoubleRow, passage-stationary):
  - Each core gets a 32768-passage shard (of 262144) plus the full 2048
    queries, both cast to fp8e4m3 host-side (fast bit-twiddling cast) and
    pre-transposed to K-major layout (25MB + 2MB per core).
  - Device: S = P_shard @ Q.T as fp8 DoubleRow matmuls with the PASSAGE
    tile stationary: per 128-passage chunk, 3 k-pair weight loads each
    streamed by 4 query blocks of 512 (12 MMs of 216ns = the fp8-DR
    streaming floor). Output [128, 2048] fp8 per pchunk is cast out of
    PSUM by ACT/DVE alternately and DMA'd every ~2.6us - a uniform
    output stream (no bursts, ~1.5us tail) vs the query-stationary
    variant's 8.4MB end-of-group bursts.
  - Host: maps fp8 sims to order-preserving uint8 keys, cuts each query
    at a sampled threshold (~1.5k survivors; the exact top-k is a subset
    with tens-of-sigma margin vs the ~1-sigma fp8 matmul noise),
    rescores every survivor exactly in fp32 (blocked BLAS GEMM), and
    takes the exact top-k with jax.lax.top_k tie-breaking (lowest index
    first).
"""
import os
import time as _time

import numpy as np

import concourse.bacc as bacc
import concourse.tile as tile
from concourse import mybir
from concourse.bass_utils import run_bass_kernel_spmd

P = 128
Q = 2048              # queries (replicated on all cores)
D = 768               # embedding dim = 6 k-tiles of 128
NCORES = 8
NTOTAL = 262144       # total passages
NSH = NTOTAL // NCORES  # 32768 passages per core
CHUNK = 512           # queries per PSUM bank (moving dim)
NQG = Q // CHUNK      # 4 query groups
NPC = NSH // P        # 256 passage chunks per core
PG = 4                # passage chunks per input DMA group
KT = D // P           # 6 k-tiles

TRACE = False         # set True (e.g. from test.py) to capture an NTFF profile
LAST_PERF = None      # BassKernelResults of the last run when TRACE was set

RESCORE8 = 1024       # target survivor count per query for the host rescore

_NC_CACHE = {}


def _build_fp8():
    FP8 = mybir.dt.float8e4
    nc = bacc.Bacc("TRN2", target_bir_lowering=False)
    qt = nc.dram_tensor("qt", [D, Q], FP8, kind="ExternalInput")
    # pt arrives pre-tiled host-side as [NG, 128, 6, PG*128] so every
    # group DMA reads one contiguous block (3KB/partition runs) instead
    # of 768 separate 1KB strided strips - 8.3us -> 1.1us per group DMA
    # on the single queue.
    pt = nc.dram_tensor("pt", [NPC // PG, P, KT, PG * P], FP8, kind="ExternalInput")
    sims = nc.dram_tensor("sims", [NSH, Q], FP8, kind="ExternalOutput")

    qt_ap = qt.ap().rearrange("(s p) q -> p s q", p=P)   # [128, 6, 2048]

    DR = mybir.MatmulPerfMode.DoubleRow
    NG = NPC // PG  # input DMA groups

    with tile.TileContext(nc) as tc:
        with (
            tc.tile_pool(name="qpool", bufs=1) as qpool,
            tc.tile_pool(name="ppool", bufs=3) as ppool,
            tc.tile_pool(name="spool", bufs=3) as spool,
            tc.tile_pool(name="pspool", bufs=8, space="PSUM") as pspool,
        ):
            # First passage tile group, then queries in 12 k-pair/column
            # pieces: the first matmul needs pt group 0 + the first
            # [128,2,512] query piece only, so it starts ~8us in instead
            # of waiting for the full 1.5MB query load on the single
            # DMA queue.
            cur = ppool.tile([P, KT, PG * P], FP8, tag="pt", name="pt_t")
            nc.sync.dma_start(cur[:], pt.ap()[0])

            qt_t = qpool.tile([P, KT, Q], FP8, name="qt_t")
            for kk in range(KT // 2):
                for j in range(NQG):
                    sl = (slice(None), slice(2 * kk, 2 * kk + 2),
                          slice(j * CHUNK, (j + 1) * CHUNK))
                    nc.sync.dma_start(qt_t[sl], qt_ap[sl])

            for g in range(NG):
                if g + 1 < NG:
                    nxt = ppool.tile([P, KT, PG * P], FP8, tag="pt", name="pt_t")
                    nc.sync.dma_start(nxt[:], pt.ap()[g + 1])
                else:
                    nxt = None
                for pi in range(PG):
                    pglob = g * PG + pi
                    st = spool.tile([P, Q], FP8, tag="st", name="st")
                    ps = [pspool.tile([P, CHUNK], mybir.dt.float32, tag="ps", name="ps")
                          for _ in range(NQG)]
                    for kk in range(KT // 2):
                        w = cur[:, 2 * kk:2 * kk + 2, pi * P:(pi + 1) * P]
                        for j in range(NQG):
                            nc.tensor.matmul(
                                ps[j][:], w,
                                qt_t[:, 2 * kk:2 * kk + 2, j * CHUNK:(j + 1) * CHUNK],
                                start=(kk == 0), stop=(kk == KT // 2 - 1),
                                perf_mode=DR,
                            )
                    for j in range(NQG):
                        dst = st[:, j * CHUNK:(j + 1) * CHUNK]
                        # alternate the PSUM->SBUF cast between ACT and DVE
                        if j % 2 == 0:
                            nc.scalar.copy(dst, ps[j][:])
                        else:
                            nc.vector.tensor_copy(dst, ps[j][:])
                    nc.sync.dma_start(
                        sims.ap()[pglob * P:(pglob + 1) * P, :], st[:])
                cur = nxt
    nc.compile()
    return nc


def _fp8_sort_keys(a):
    """Order-preserving fp8 -> uint8 map (no NaNs expected)."""
    u = a.view(np.uint8)
    flip = (u >> 7) * np.uint8(0x7F) + np.uint8(0x80)
    return u ^ flip


def _t(label, t0):
    if os.environ.get("KNN_TIMING"):
        print(f"[knn] {label}: {_time.time() - t0:.2f}s", flush=True)
    return _time.time()


def kernel(query_embed, passage_embed, top_k):
    global LAST_PERF, _NC_CACHE
    t0 = _time.time()
    q = np.ascontiguousarray(np.asarray(query_embed, dtype=np.float32))
    p = np.asarray(passage_embed, dtype=np.float32)
    k = int(top_k)
    assert q.shape == (Q, D) and p.shape == (NTOTAL, D), (q.shape, p.shape)
    assert 1 <= k <= 128, k

    if "fp8dr" not in _NC_CACHE:
        _NC_CACHE["fp8dr"] = _build_fp8()
    nc = _NC_CACHE["fp8dr"]
    t0 = _t("build", t0)

    NP8 = mybir.dt.np(mybir.dt.float8e4)
    q8 = q.astype(NP8)
    p8u = p.astype(NP8).view(np.uint8)
    qt = np.ascontiguousarray(q8.T)
    # device pt layout: [NG, 128, 6, PG*128] with
    # pt[g, pp, s, gi*128 + c] = shard[(g*PG + gi)*128 + c, s*128 + pp]
    in_maps = []
    for c in range(NCORES):
        sh = p8u[c * NSH:(c + 1) * NSH]                 # [32768, 768]
        ptt = np.ascontiguousarray(
            sh.reshape(NPC // PG, PG * P, KT, P).transpose(0, 3, 2, 1)
        ).view(NP8)
        in_maps.append({"qt": qt, "pt": ptt})
    t0 = _t("input prep", t0)
    out = run_bass_kernel_spmd(nc, in_maps, core_ids=list(range(NCORES)), trace=TRACE)
    if TRACE:
        LAST_PERF = out
    t0 = _t("device run (incl neff compile + transfers)", t0)

    # sims arrive [NSH, Q] per core (passage-major). Work per-core to
    # avoid a 536MB host transpose.
    keys_list = [_fp8_sort_keys(np.asarray(out.results[c]["sims"]))
                 for c in range(NCORES)]
    t0 = _t("keys", t0)
    # Per-query key threshold from a passage sample (first 16384 passages
    # = first half of core 0's shard), aiming for ~1.5*RESCORE8
    # survivors; statistically the true top-k is always a subset (fp8
    # matmul noise sigma ~1, fp8 key buckets ~8 wide at the boundary, vs
    # tens-of-sigma margins in the cut). Queries whose survivor count
    # lands low get an exact top-RESCORE8 fallback.
    m = RESCORE8
    S = NTOTAL // 16
    skth = (3 * m) // (2 * 16)
    th = np.partition(keys_list[0][:S], S - skth, axis=0)[S - skth]  # [Q]
    masks = [kk >= th[None, :] for kk in keys_list]
    counts = masks[0].sum(axis=0, dtype=np.int32)
    for mk in masks[1:]:
        counts += mk.sum(axis=0, dtype=np.int32)
    bad = np.nonzero((counts < max(2 * k, 256)) | (counts > 16 * m))[0]
    for r in bad:  # rare (sampling tail); re-cut the query at its exact
        # m-th largest key, tie-inclusive so equal keys are all kept
        col = np.concatenate([kk[:, r] for kk in keys_list])
        th_r = np.partition(col, NTOTAL - m)[NTOTAL - m]
        for c in range(NCORES):
            masks[c][:, r] = keys_list[c][:, r] >= th_r
    t0 = _t("threshold scan", t0)
    key_parts = []
    for c in range(NCORES):
        pr, qr = np.nonzero(masks[c])
        # composite sort key: query (11 bits) then global passage (18 bits)
        key_parts.append((qr << 18) | (pr + c * NSH))
    skey = np.concatenate(key_parts)
    skey.sort()
    rows = (skey >> 18).astype(np.int32)
    cols = (skey & ((1 << 18) - 1)).astype(np.int32)
    row_starts = np.searchsorted(rows, np.arange(Q + 1))
    t0 = _t("survivors", t0)
    # exact fp32 rescore of every survivor: blocked GEMM over query
    # stripes (gather survivor passages once, multiply by all QB queries,
    # select the matching column - BLAS-fast despite the QB-x overcompute)
    exact = np.empty(len(cols), dtype=np.float32)
    QB = 16
    for r0 in range(0, Q, QB):
        s0, e0 = row_starts[r0], row_starts[r0 + QB]
        if e0 == s0:
            continue
        sb = p[cols[s0:e0]] @ q[r0:r0 + QB].T          # [ns, QB]
        exact[s0:e0] = sb[np.arange(e0 - s0), rows[s0:e0] - r0]
    t0 = _t("rescore", t0)
    # ties -> lowest passage index, matching jax.lax.top_k
    order = np.lexsort((cols, -exact, rows))
    cols = cols[order]
    exact = exact[order]
    pick = (row_starts[:-1, None] + np.arange(k)[None, :]).ravel()
    inds = cols[pick].reshape(Q, k).astype(np.int32)
    vals = exact[pick].reshape(Q, k)
    t0 = _t("final sort", t0)
    return inds, vals


# revision 9
# speedup vs baseline: 1.0445x; 1.0022x over previous
"""Sharded brute-force kNN (cosine-sim top-k) on 8 Trainium2 NeuronCores.

Strategy (passage-row-wise sharding, fp8 DoubleRow, passage-stationary):
  - Each core gets a 32768-passage shard (of 262144) plus the full 2048
    queries, both cast to fp8e4m3 host-side (fast bit-twiddling cast) and
    pre-transposed to K-major layout (25MB + 2MB per core).
  - Device: S = P_shard @ Q.T as fp8 DoubleRow matmuls with the PASSAGE
    tile stationary: per 128-passage chunk, 3 k-pair weight loads each
    streamed by 4 query blocks of 512 (12 MMs of 216ns = the fp8-DR
    streaming floor). Output [128, 2048] fp8 per pchunk is cast out of
    PSUM by ACT/DVE alternately and DMA'd every ~2.6us - a uniform
    output stream (no bursts, ~1.5us tail) vs the query-stationary
    variant's 8.4MB end-of-group bursts.
  - Host: maps fp8 sims to order-preserving uint8 keys, cuts each query
    at a sampled threshold (~1.5k survivors; the exact top-k is a subset
    with tens-of-sigma margin vs the ~1-sigma fp8 matmul noise),
    rescores every survivor exactly in fp32 (blocked BLAS GEMM), and
    takes the exact top-k with jax.lax.top_k tie-breaking (lowest index
    first).
"""
import os
import time as _time

import numpy as np

import concourse.bacc as bacc
import concourse.tile as tile
from concourse import mybir
from concourse.bass_utils import run_bass_kernel_spmd

P = 128
Q = 2048              # queries (replicated on all cores)
D = 768               # embedding dim = 6 k-tiles of 128
NCORES = 8
NTOTAL = 262144       # total passages
NSH = NTOTAL // NCORES  # 32768 passages per core
CHUNK = 512           # queries per PSUM bank (moving dim)
NQG = Q // CHUNK      # 4 query groups
NPC = NSH // P        # 256 passage chunks per core
PG = 4                # passage chunks per input DMA group
KT = D // P           # 6 k-tiles

TRACE = False         # set True (e.g. from test.py) to capture an NTFF profile
LAST_PERF = None      # BassKernelResults of the last run when TRACE was set

RESCORE8 = 1024       # target survivor count per query for the host rescore

_NC_CACHE = {}


def _build_fp8():
    FP8 = mybir.dt.float8e4
    nc = bacc.Bacc("TRN2", target_bir_lowering=False)
    qt = nc.dram_tensor("qt", [D, Q], FP8, kind="ExternalInput")
    # pt arrives pre-tiled host-side as [NG, 128, 6, PG*128] so every
    # group DMA reads one contiguous block (3KB/partition runs) instead
    # of 768 separate 1KB strided strips - 8.3us -> 1.1us per group DMA
    # on the single queue.
    pt = nc.dram_tensor("pt", [NPC // PG, P, KT, PG * P], FP8, kind="ExternalInput")
    sims = nc.dram_tensor("sims", [NSH, Q], FP8, kind="ExternalOutput")

    qt_ap = qt.ap().rearrange("(s p) q -> p s q", p=P)   # [128, 6, 2048]

    DR = mybir.MatmulPerfMode.DoubleRow
    NG = NPC // PG  # input DMA groups

    with tile.TileContext(nc) as tc:
        with (
            tc.tile_pool(name="qpool", bufs=1) as qpool,
            tc.tile_pool(name="ppool", bufs=3) as ppool,
            tc.tile_pool(name="spool", bufs=3) as spool,
            tc.tile_pool(name="pspool", bufs=8, space="PSUM") as pspool,
        ):
            # First passage tile group, then queries in 12 k-pair/column
            # pieces: the first matmul needs pt group 0 + the first
            # [128,2,512] query piece only, so it starts ~8us in instead
            # of waiting for the full 1.5MB query load on the single
            # DMA queue.
            cur = ppool.tile([P, KT, PG * P], FP8, tag="pt", name="pt_t")
            nc.sync.dma_start(cur[:], pt.ap()[0])

            qt_t = qpool.tile([P, KT, Q], FP8, name="qt_t")
            for kk in range(KT // 2):
                for j in range(NQG):
                    sl = (slice(None), slice(2 * kk, 2 * kk + 2),
                          slice(j * CHUNK, (j + 1) * CHUNK))
                    nc.sync.dma_start(qt_t[sl], qt_ap[sl])

            for g in range(NG):
                if g + 1 < NG:
                    nxt = ppool.tile([P, KT, PG * P], FP8, tag="pt", name="pt_t")
                    nc.sync.dma_start(nxt[:], pt.ap()[g + 1])
                else:
                    nxt = None
                for pi in range(PG):
                    pglob = g * PG + pi
                    st = spool.tile([P, Q], FP8, tag="st", name="st")
                    ps = [pspool.tile([P, CHUNK], mybir.dt.float32, tag="ps", name="ps")
                          for _ in range(NQG)]
                    for kk in range(KT // 2):
                        w = cur[:, 2 * kk:2 * kk + 2, pi * P:(pi + 1) * P]
                        for j in range(NQG):
                            nc.tensor.matmul(
                                ps[j][:], w,
                                qt_t[:, 2 * kk:2 * kk + 2, j * CHUNK:(j + 1) * CHUNK],
                                start=(kk == 0), stop=(kk == KT // 2 - 1),
                                perf_mode=DR,
                            )
                    for j in range(NQG):
                        dst = st[:, j * CHUNK:(j + 1) * CHUNK]
                        # alternate the PSUM->SBUF cast between ACT and DVE
                        if j % 2 == 0:
                            nc.scalar.copy(dst, ps[j][:])
                        else:
                            nc.vector.tensor_copy(dst, ps[j][:])
                    nc.sync.dma_start(
                        sims.ap()[pglob * P:(pglob + 1) * P, :], st[:])
                cur = nxt
    nc.compile()
    return nc


def _fp8_sort_keys(a):
    """Order-preserving fp8 -> uint8 map (no NaNs expected)."""
    u = a.view(np.uint8)
    flip = (u >> 7) * np.uint8(0x7F) + np.uint8(0x80)
    return u ^ flip


def _t(label, t0):
    if os.environ.get("KNN_TIMING"):
        print(f"[knn] {label}: {_time.time() - t0:.2f}s", flush=True)
    return _time.time()


def kernel(query_embed, passage_embed, top_k):
    global LAST_PERF, _NC_CACHE
    t0 = _time.time()
    q = np.ascontiguousarray(np.asarray(query_embed, dtype=np.float32))
    p = np.asarray(passage_embed, dtype=np.float32)
    k = int(top_k)
    assert q.shape == (Q, D) and p.shape == (NTOTAL, D), (q.shape, p.shape)
    assert 1 <= k <= 128, k

    if "fp8dr" not in _NC_CACHE:
        _NC_CACHE["fp8dr"] = _build_fp8()
    nc = _NC_CACHE["fp8dr"]
    t0 = _t("build", t0)

    NP8 = mybir.dt.np(mybir.dt.float8e4)
    q8 = q.astype(NP8)
    p8u = p.astype(NP8).view(np.uint8)
    qt = np.ascontiguousarray(q8.T)
    # device pt layout: [NG, 128, 6, PG*128] with
    # pt[g, pp, s, gi*128 + c] = shard[(g*PG + gi)*128 + c, s*128 + pp]
    in_maps = []
    for c in range(NCORES):
        sh = p8u[c * NSH:(c + 1) * NSH]                 # [32768, 768]
        ptt = np.ascontiguousarray(
            sh.reshape(NPC // PG, PG * P, KT, P).transpose(0, 3, 2, 1)
        ).view(NP8)
        in_maps.append({"qt": qt, "pt": ptt})
    t0 = _t("input prep", t0)
    out = run_bass_kernel_spmd(nc, in_maps, core_ids=list(range(NCORES)), trace=TRACE)
    if TRACE:
        LAST_PERF = out
    t0 = _t("device run (incl neff compile + transfers)", t0)

    # sims arrive [NSH, Q] per core (passage-major). Work per-core to
    # avoid a 536MB host transpose.
    keys_list = [_fp8_sort_keys(np.asarray(out.results[c]["sims"]))
                 for c in range(NCORES)]
    t0 = _t("keys", t0)
    # Per-query key threshold from a passage sample (first 16384 passages
    # = first half of core 0's shard), aiming for ~1.5*RESCORE8
    # survivors; statistically the true top-k is always a subset (fp8
    # matmul noise sigma ~1, fp8 key buckets ~8 wide at the boundary, vs
    # tens-of-sigma margins in the cut). Queries whose survivor count
    # lands low get an exact top-RESCORE8 fallback.
    m = RESCORE8
    S = NTOTAL // 16
    skth = (3 * m) // (2 * 16)
    th = np.partition(keys_list[0][:S], S - skth, axis=0)[S - skth]  # [Q]
    t0 = _t("  th partition", t0)
    masks = [kk >= th[None, :] for kk in keys_list]
    t0 = _t("  masks", t0)
    counts = masks[0].sum(axis=0, dtype=np.int32)
    for mk in masks[1:]:
        counts += mk.sum(axis=0, dtype=np.int32)
    t0 = _t("  counts", t0)
    bad = np.nonzero((counts < max(2 * k, 256)) | (counts > 16 * m))[0]
    if os.environ.get("KNN_TIMING"):
        print(f"[knn]   bad rows: {len(bad)}; counts min/med/max: "
              f"{counts.min()}/{int(np.median(counts))}/{counts.max()}", flush=True)
    for r in bad:  # rare (sampling tail); re-cut the query at its exact
        # m-th largest key, tie-inclusive so equal keys are all kept
        col = np.concatenate([kk[:, r] for kk in keys_list])
        th_r = np.partition(col, NTOTAL - m)[NTOTAL - m]
        for c in range(NCORES):
            masks[c][:, r] = keys_list[c][:, r] >= th_r
    t0 = _t("threshold scan", t0)
    key_parts = []
    for c in range(NCORES):
        pr, qr = np.nonzero(masks[c])
        # composite sort key: query (11 bits) then global passage (18 bits)
        key_parts.append((qr << 18) | (pr + c * NSH))
    skey = np.concatenate(key_parts)
    skey.sort()
    rows = (skey >> 18).astype(np.int32)
    cols = (skey & ((1 << 18) - 1)).astype(np.int32)
    row_starts = np.searchsorted(rows, np.arange(Q + 1))
    t0 = _t("survivors", t0)
    # exact fp32 rescore of every survivor: blocked GEMM over query
    # stripes (gather survivor passages once, multiply by all QB queries,
    # select the matching column - BLAS-fast despite the QB-x overcompute)
    exact = np.empty(len(cols), dtype=np.float32)
    QB = 16
    for r0 in range(0, Q, QB):
        s0, e0 = row_starts[r0], row_starts[r0 + QB]
        if e0 == s0:
            continue
        sb = p[cols[s0:e0]] @ q[r0:r0 + QB].T          # [ns, QB]
        exact[s0:e0] = sb[np.arange(e0 - s0), rows[s0:e0] - r0]
    t0 = _t("rescore", t0)
    # ties -> lowest passage index, matching jax.lax.top_k
    order = np.lexsort((cols, -exact, rows))
    cols = cols[order]
    exact = exact[order]
    pick = (row_starts[:-1, None] + np.arange(k)[None, :]).ravel()
    inds = cols[pick].reshape(Q, k).astype(np.int32)
    vals = exact[pick].reshape(Q, k)
    t0 = _t("final sort", t0)
    return inds, vals
